# revision 11
# baseline (speedup 1.0000x reference)
"""BinaryOneToManyMatcher (nms_detection) Trainium2 Bass kernel.

Computes, for B=128 images with Q=1000 predicted boxes and G=300 GT boxes:
  score = sigmoid(pred_logits)            [B,Q]
  iou   = pairwise IoU(pred, tgt)         [B,Q,G]
  gt    = score * iou * (iou > 0.4)       [B,Q,G]
  vals, idxs = top_k(gt over Q, k=4); mask = vals > 0

Sharding: pure data parallel, 16 images per NeuronCore across 8 cores.

Per-core layout: per image, G on partitions in 3 chunks of 100 and Q on the
free dim (1000 wide).  Per-query rows (x1,y1,x2,y2,area+eps,score) are
broadcast across partitions via PE ones-matmul (bit-exact); per-target
values are [P,1] per-partition scalars.

v2 changes vs the original baseline (1.04ms HW):
 - DMA count per core cut from 436 to ~25.  Each DMA serializes ~0.6us on
   the shared HWDGE unit plus ~1us of descriptor/semaphore latency, so the
   baseline spent ~450us on DMA.  Now: queries are packed once into a
   per-image "line" layout [16,6000] via one SBUF->SBUF reorg DMA, each
   image stages its 6 rows with ONE single-descriptor DMA, target boxes
   arrive in one batched DMA, target areas are computed on-chip, and the
   three outputs are written with one batched DMA each at the end.
 - Mask chain restructured: valid = inter > 0.4*Up computed as a single
   fused custom-DVE select (MASKNUM) instead of 4 gpsimd + 1 act ops.
 - Epilogue (zeroing + bool mask) runs once per core instead of per image.

Top-4 uses the DVE Max8 instruction (top-8 per partition, descending) +
MaxIndex.  A strictly-decreasing per-q bias of scale 2^-40 is added to the
masked scores so zero entries (invalid pairs) sort by ascending q, matching
jax.lax.top_k's lowest-index-first tie rule; the bias is far below the
minimum positive score gap so positive ordering is unchanged.
"""

import os
from contextlib import ExitStack

import numpy as np

import concourse.bass as bass
import concourse.tile as tile
from concourse import bacc, mybir
from concourse.bass_utils import run_bass_kernel_spmd

B, Q, G, K = 128, 1000, 300, 4
NCORES = 8
BPC = B // NCORES  # images per core
PCH = 100          # partitions per g-chunk (3 chunks of 100 = G)
NCH = G // PCH

F32 = mybir.dt.float32
I32 = mybir.dt.int32
U32 = mybir.dt.uint32
U8 = mybir.dt.uint8
Op = mybir.AluOpType

BIAS_SCALE = float(2.0**-40)  # per-q tie-break bias scale
POS_THRESH = 1e-6  # separates real positives (>=3e-3) from bias values (<1e-9)


def _register_dve_ops():
    """Custom DVE ops, each one full-rate pass:

    WSUB_ANT:   out = min(in0, s0) - max(in1, s1)       (overlap width)
    MASKNUM_ANT: out = in0 if in0 > in1*s0 else 0       (masked numerator)
    """
    from concourse import dve_ops
    from concourse.dve_spec import (
        Spec, Src0, Src1, C0, C1, Zero, Idx, minn, maxx, select, relu, lower,
    )
    from concourse.dve_uop import DveOpSpec

    def reg(name, spec):
        for op in dve_ops.OPS:
            if op.name == name:
                return op
        shas = {}
        for ver in ("v3", "v4"):
            try:
                uops = lower(spec, ver=ver)
                shas[ver] = DveOpSpec(
                    name=name, opcode=0, uops=uops, rd1_en=True
                ).sha(ver)
            except Exception:
                pass
        op = dve_ops.DveOp(name, spec, subdim=False, uops_sha=shas)
        dve_ops.OPS.append(op)
        dve_ops.CUSTOM_DVE_SPECS[op.name] = spec
        dve_ops._SUB_OPCODE_FOR_NAME[op.name] = (
            max(dve_ops._SUB_OPCODE_FOR_NAME.values()) + 1
        )
        assert dve_ops._SUB_OPCODE_FOR_NAME[op.name] < 0x20
        return op

    wsub = reg("WSUB_ANT", Spec(
        body=minn(Src0, C0) - maxx(Src1, C1),
        reference=lambda in0, in1, s0, s1, imm2: (
            np.minimum(in0.astype(np.float32), s0) - np.maximum(in1, s1)
        ).astype(np.float32),
    ))
    wsubrelu = reg("WSUBRELU_ANT", Spec(
        body=relu(minn(Src0, C0) - maxx(Src1, C1)),
        reference=lambda in0, in1, s0, s1, imm2: np.maximum(
            np.minimum(in0.astype(np.float32), s0) - np.maximum(in1, s1), 0.0
        ).astype(np.float32),
    ))
    masknum = reg("MASKNUM_ANT", Spec(
        body=select(Src0 > Src1 * C0, Src0, Zero),
        reference=lambda in0, in1, s0, s1, imm2: np.where(
            in0 > (in1 * s0).astype(np.float32), in0, np.float32(0.0)
        ).astype(np.float32),
    ))

    def _ioumask_ref(in0, in1, s0, s1, imm2):
        iou = (in0 * in1).astype(np.float32)
        return np.where(iou > s0, iou, np.float32(0.0)).astype(np.float32)

    ioumask = reg("IOUMASK_ANT", Spec(
        body=select(Src0 * Src1 > C0, Src0 * Src1, Zero),
        reference=_ioumask_ref,
    ))

    def _valbias_ref(in0, in1, s0, s1, imm2):
        q = np.arange(in0.shape[-1], dtype=np.float32)
        return ((in0 * in1).astype(np.float32)
                + (s0 - q * s1).astype(np.float32)).astype(np.float32)

    valbias = reg("VALBIAS_ANT", Spec(
        body=Src0 * Src1 + (C0 - Idx * C1),
        reference=_valbias_ref,
    ))
    return wsub, wsubrelu, masknum, ioumask, valbias


def _build_kernel(reps=1):
    wsub, wsubrelu, masknum, ioumask, valbias = _register_dve_ops()
    from concourse.dve_ops import RECIPROCAL_APPROX_NR

    kb_inter = os.environ.get("KB_INTER", "pool")
    kb_up = os.environ.get("KB_UP", "actpool")
    kb_mask = os.environ.get("KB_MASK", "fused")
    kb_vb = os.environ.get("KB_VB", "fused")
    kb_recip = os.environ.get("KB_RECIP", "accurate")
    kb_out = os.environ.get("KB_OUT", "batch")

    nc = bacc.Bacc("TRN2", target_bir_lowering=False, debug=False,
                   num_devices=NCORES)

    pl = nc.dram_tensor("pred_logits", [BPC, Q, 1], F32, kind="ExternalInput").ap()
    pb = nc.dram_tensor("pred_boxes", [BPC, Q, 4], F32, kind="ExternalInput").ap()
    tb = nc.dram_tensor("tgt_boxes", [BPC, G, 4], F32, kind="ExternalInput").ap()

    vals_o = nc.dram_tensor("vals", [BPC, G, K], F32, kind="ExternalOutput").ap()
    idxs_o = nc.dram_tensor("idxs", [BPC, G, K], I32, kind="ExternalOutput").ap()
    mask_o = nc.dram_tensor("mask", [BPC, G, K], U8, kind="ExternalOutput").ap()

    PH = 8          # partitions per image in the packed query layout
    QP = Q // PH    # 125 queries per partition

    with tile.TileContext(nc) as tc, ExitStack() as ctx:
        const = ctx.enter_context(tc.tile_pool(name="const", bufs=1))
        prep = ctx.enter_context(tc.tile_pool(name="prep", bufs=1))
        persist = ctx.enter_context(tc.tile_pool(name="persist", bufs=1))
        stagep = ctx.enter_context(tc.tile_pool(name="stage", bufs=2))
        rows = ctx.enter_context(tc.tile_pool(name="rows", bufs=2))
        work = ctx.enter_context(tc.tile_pool(name="work", bufs=2))
        psum = ctx.enter_context(tc.tile_pool(name="psum", bufs=4, space="PSUM"))

        # ---- constants
        ones = const.tile([1, 128], F32, tag="ones")
        nc.vector.memset(ones[:], 1.0)
        # tie-break bias row: (Q - q) * 2^-40, identical on all partitions
        bias_i = const.tile([128, Q], I32, tag="bias_i")
        nc.gpsimd.iota(bias_i[:], pattern=[[-1, Q]], base=Q, channel_multiplier=0)
        bias_f = const.tile([128, Q], F32, tag="bias_f")
        nc.vector.tensor_scalar(bias_f[:], bias_i[:], BIAS_SCALE, None, Op.mult)


        # ---- prep: pack per-query rows into per-image lines [16, 6000]
        # lines_all[b, :] = [px1|py1|px2|py2 (ph,c,r packed), pa+eps, score]
        lines_all = persist.tile([BPC, 6 * Q], F32, tag="lines")

        pbt = prep.tile([128, QP * 4], F32, tag="pbt")
        nc.sync.dma_start(
            pbt[:],
            pb.rearrange("b q c -> (b q c)").rearrange("(p x) -> p x", p=128),
        )
        # free layout (r,c) -> (c,r) so coord rows are contiguous per partition
        pbt2 = prep.tile([128, QP * 4], F32, tag="pbt2")
        nc.vector.tensor_scalar(
            pbt2[:].rearrange("p (c r) -> p c r", c=4),
            pbt[:].rearrange("p (r c) -> p r c", c=4).transpose([0, 2, 1]),
            0.0, None, Op.add
        )
        dx = prep.tile([128, QP], F32, tag="dx")
        dy = prep.tile([128, QP], F32, tag="dy")
        pa0 = prep.tile([128, QP], F32, tag="pa0")
        paE = prep.tile([128, QP], F32, tag="paE")
        nc.vector.tensor_tensor(dx[:], pbt2[:, 2 * QP:3 * QP], pbt2[:, 0:QP],
                                Op.subtract)
        nc.vector.tensor_tensor(dy[:], pbt2[:, 3 * QP:4 * QP], pbt2[:, QP:2 * QP],
                                Op.subtract)
        nc.vector.tensor_tensor(pa0[:], dx[:], dy[:], Op.mult)
        # fold the union's +1e-7 into the query area (union = pa+eps+ta-inter)
        nc.vector.tensor_scalar(paE[:], pa0[:], 1e-7, None, Op.add)

        # sigmoid(x) = 1 / (1 + exp(-x)); exp on ScalarE, accurate recip on DVE
        lg = prep.tile([128, QP], F32, tag="lg")
        nc.sync.dma_start(
            lg[:], pl.rearrange("b q c -> (b q c)").rearrange("(p x) -> p x", p=128)
        )
        ex = prep.tile([128, QP], F32, tag="ex")
        nc.scalar.activation(ex[:], lg[:], mybir.ActivationFunctionType.Exp,
                             scale=-1.0)
        w1 = prep.tile([128, QP], F32, tag="w1")
        nc.vector.tensor_scalar(w1[:], ex[:], 1.0, None, Op.add)
        sc = prep.tile([128, QP], F32, tag="sc")
        scr = prep.tile([128, QP], F32, tag="scr")
        nc.vector.reciprocal_approx_accurate(sc[:], w1[:], scr[:])

        # one reorg DMA each: [128, x] query-packed -> [16, 8x] image-packed
        nc.sync.dma_start(lines_all[:, 0:4 * Q], pbt2[:])
        nc.sync.dma_start(lines_all[:, 4 * Q:5 * Q], paE[:])
        nc.sync.dma_start(lines_all[:, 5 * Q:6 * Q], sc[:])

        # ---- prep: all target boxes in one DMA; areas computed on-chip
        # tsc_all[p, (b,c,k)] = tgt box k-coord of gt (c*100+p) of image b
        tsc_all = persist.tile([PCH, BPC * NCH * 4], F32, tag="tsc")
        nc.sync.dma_start(
            tsc_all[:], tb.rearrange("b (c p) k -> p b c k", c=NCH, p=PCH)
        )
        ta_all = persist.tile([PCH, BPC * NCH], F32, tag="ta")
        tdx = prep.tile([PCH, BPC * NCH], F32, tag="tdx")
        tdy = prep.tile([PCH, BPC * NCH], F32, tag="tdy")
        tv = tsc_all[:].rearrange("p (s k) -> p s k", k=4)
        nc.vector.tensor_tensor(tdx[:], tv[:, :, 2], tv[:, :, 0], Op.subtract)
        nc.vector.tensor_tensor(tdy[:], tv[:, :, 3], tv[:, :, 1], Op.subtract)
        nc.vector.tensor_tensor(ta_all[:], tdx[:], tdy[:], Op.mult)

        # ---- collectors for the whole core (written per chunk, drained once)
        v8all = persist.tile([PCH, BPC * NCH * 8], F32, tag="v8all")
        i8all = persist.tile([PCH, BPC * NCH * 8], U32, tag="i8all")
        vals4 = persist.tile([PCH, BPC * NCH * K], F32, tag="vals4")
        mask4 = persist.tile([PCH, BPC * NCH * K], U8, tag="mask4")

        HB = 500  # psum bank-sized matmul piece (N<=512)

        for _ in range(reps):
            for b in range(BPC):
                # stage this image's 6 rows on partition 0 (single-descriptor)
                stage = stagep.tile([1, 6 * Q], F32, tag="stage")
                nc.sync.dma_start(stage[:], lines_all[b:b + 1, :])
                boxv = stage[:, 0:4 * Q].rearrange(
                    "o (ph c r) -> o ph c r", ph=PH, c=4)
                pav = stage[:, 4 * Q:5 * Q].rearrange("o (ph r) -> o ph r", ph=PH)
                scv = stage[:, 5 * Q:6 * Q].rearrange("o (ph r) -> o ph r", ph=PH)

                # PE ones-matmul broadcast (bit-exact 1.0*x) + ScalarE copies
                r_px1 = rows.tile([128, Q], F32, tag="px1")
                r_py1 = rows.tile([128, Q], F32, tag="py1")
                r_px2 = rows.tile([128, Q], F32, tag="px2")
                r_py2 = rows.tile([128, Q], F32, tag="py2")
                r_pa = rows.tile([128, Q], F32, tag="pa")
                r_sc = rows.tile([128, Q], F32, tag="sc")
                views = [boxv[:, :, 0, :], boxv[:, :, 1, :], boxv[:, :, 2, :],
                         boxv[:, :, 3, :], pav, scv]
                for rt, view in zip((r_px1, r_py1, r_px2, r_py2, r_pa, r_sc),
                                    views):
                    pt = psum.tile([128, 1024], F32, tag="pt")
                    for h in range(2):
                        nc.tensor.matmul(
                            pt[:, h * 512:h * 512 + HB], ones[:],
                            view[:, 4 * h:4 * h + 4, :],
                            start=True, stop=True)
                    nc.scalar.activation(
                        rt[:].rearrange("p (h x) -> p h x", h=2),
                        pt[:].rearrange("p (h x) -> p h x", h=2)[:, :, 0:HB],
                        mybir.ActivationFunctionType.Copy)

                for c in range(NCH):
                    sb = b * NCH + c
                    ts4 = tsc_all[0:PCH, 4 * sb:4 * sb + 4]
                    tx1, ty1 = ts4[:, 0:1], ts4[:, 1:2]
                    tx2, ty2 = ts4[:, 2:3], ts4[:, 3:4]
                    ta = ta_all[0:PCH, sb:sb + 1]

                    # overlap widths; relu folded into the x op so inter
                    # is a plain multiply
                    wxr = work.tile([PCH, Q], F32, tag="A")
                    nc.vector._custom_dve(wsubrelu, out=wxr[:], in0=r_px2[0:PCH],
                                          in1=r_px1[0:PCH], s0=tx2, s1=tx1)
                    wyr = work.tile([PCH, Q], F32, tag="B")
                    nc.vector._custom_dve(wsub, out=wyr[:], in0=r_py2[0:PCH],
                                          in1=r_py1[0:PCH], s0=ty2, s1=ty1)
                    # inter = relu(wxr) * wyr (sign-exact where it matters);
                    # the one op left on Pool -- it overlaps under DVE work
                    inter = work.tile([PCH, Q], F32, tag="C")
                    if kb_inter == "pool":
                        nc.gpsimd.tensor_tensor(inter[:], wxr[:], wyr[:], Op.mult)
                    else:
                        nc.vector.tensor_tensor(inter[:], wxr[:], wyr[:], Op.mult)
                    # Up = (pa+eps + ta) - inter
                    Up = work.tile([PCH, Q], F32, tag="D")
                    if kb_up == "actpool":
                        srow = work.tile([PCH, Q], F32, tag="H")
                        nc.scalar.activation(srow[:], r_pa[0:PCH],
                                             mybir.ActivationFunctionType.Identity,
                                             bias=ta)
                        nc.gpsimd.tensor_tensor(Up[:], srow[:], inter[:],
                                                Op.subtract)
                    else:
                        nc.vector.scalar_tensor_tensor(Up[:], r_pa[0:PCH], ta,
                                                       inter[:], Op.add,
                                                       Op.subtract)
                    # R ~= 1/Up (fast: ~51 ULP in one op; accurate: +1
                    # Newton step for ~2 ULP)
                    if kb_recip == "fast":
                        R = work.tile([PCH, Q], F32, tag="F")
                        nc.vector.reciprocal_approx_fast(out=R[:], in_=Up[:])
                    else:
                        R0 = work.tile([PCH, Q], F32, tag="E")
                        nc.vector.reciprocal_approx_fast(out=R0[:], in_=Up[:])
                        R = work.tile([PCH, Q], F32, tag="F")
                        nc.vector._custom_dve(RECIPROCAL_APPROX_NR, out=R[:],
                                              in0=Up[:], in1=R0[:], s0=2.0)
                    # A = iou if iou > 0.4 else 0, iou = inter*R  (one fused op)
                    A = work.tile([PCH, Q], F32, tag="G")
                    if kb_mask == "fused":
                        nc.vector._custom_dve(ioumask, out=A[:], in0=inter[:],
                                              in1=R[:], s0=0.4)
                    else:
                        nm = work.tile([PCH, Q], F32, tag="A")
                        nc.vector._custom_dve(masknum, out=nm[:], in0=inter[:],
                                              in1=Up[:], s0=0.4)
                        nc.gpsimd.tensor_tensor(A[:], nm[:], R[:], Op.mult)
                    # m3 = A*score + (Q - q)*2^-40   (one fused op via Idx)
                    m3 = work.tile([PCH, Q], F32, tag="B")
                    if kb_vb == "fused":
                        nc.vector._custom_dve(valbias, out=m3[:], in0=A[:],
                                              in1=r_sc[0:PCH],
                                              s0=float(Q) * BIAS_SCALE,
                                              s1=BIAS_SCALE)
                    else:
                        t1 = work.tile([PCH, Q], F32, tag="E")
                        nc.gpsimd.tensor_tensor(t1[:], A[:], r_sc[0:PCH], Op.mult)
                        nc.gpsimd.tensor_tensor(m3[:], t1[:], bias_f[0:PCH],
                                                Op.add)
                    v8 = v8all[0:PCH, 8 * sb:8 * sb + 8]
                    nc.vector.max(v8, m3[:])
                    nc.vector.max_index(i8all[0:PCH, 8 * sb:8 * sb + 8], v8, m3[:])

            # ---- epilogue: exact zeros for padding slots + bool mask
            v8v = v8all[0:PCH, :].rearrange("p (s e) -> p s e", e=8)[:, :, 0:K]
            nc.vector.scalar_tensor_tensor(
                vals4[0:PCH, :].rearrange("p (s e) -> p s e", e=K),
                v8v, POS_THRESH, v8v, Op.is_gt, Op.mult)
            nc.vector.tensor_scalar(
                mask4[0:PCH, :].rearrange("p (s e) -> p s e", e=K),
                v8v, POS_THRESH, None, Op.is_gt)

            if kb_out == "batch":
                nc.sync.dma_start(
                    vals_o.rearrange("b (c p) k -> p b c k", c=NCH, p=PCH),
                    vals4[0:PCH, :])
                nc.sync.dma_start(
                    idxs_o.rearrange("b (c p) k -> p b c k", c=NCH, p=PCH),
                    i8all[0:PCH, :].rearrange("p (s e) -> p s e", e=8)[:, :, 0:K]
                    .bitcast(I32))
                nc.sync.dma_start(
                    mask_o.rearrange("b (c p) k -> p b c k", c=NCH, p=PCH),
                    mask4[0:PCH, :])
            else:
                for b in range(BPC):
                    for c in range(NCH):
                        sb = b * NCH + c
                        g0 = c * PCH
                        nc.sync.dma_start(
                            vals_o[b, g0:g0 + PCH, :],
                            vals4[0:PCH, K * sb:K * sb + K])
                        nc.sync.dma_start(
                            idxs_o[b, g0:g0 + PCH, :],
                            i8all[0:PCH, 8 * sb:8 * sb + K].bitcast(I32))
                        nc.sync.dma_start(
                            mask_o[b, g0:g0 + PCH, :],
                            mask4[0:PCH, K * sb:K * sb + K])

    nc.compile()
    return nc


_NC = None


def _get_nc():
    global _NC
    if _NC is None:
        _NC = _build_kernel()
    return _NC


def run(pred_logits, pred_boxes_xyxy, tgt_boxes_xyxy, **spmd_kwargs):
    nc = _get_nc()
    pred_logits = np.ascontiguousarray(np.asarray(pred_logits, dtype=np.float32))
    pred_boxes = np.ascontiguousarray(np.asarray(pred_boxes_xyxy, dtype=np.float32))
    tgt_boxes = np.ascontiguousarray(np.asarray(tgt_boxes_xyxy, dtype=np.float32))
    in_maps = [
        {
            "pred_logits": pred_logits[c * BPC:(c + 1) * BPC],
            "pred_boxes": pred_boxes[c * BPC:(c + 1) * BPC],
            "tgt_boxes": tgt_boxes[c * BPC:(c + 1) * BPC],
        }
        for c in range(NCORES)
    ]
    res = run_bass_kernel_spmd(nc, in_maps, list(range(NCORES)), **spmd_kwargs)
    vals = np.concatenate([res.results[c]["vals"] for c in range(NCORES)], axis=0)
    idxs = np.concatenate([res.results[c]["idxs"] for c in range(NCORES)], axis=0)
    mask = np.concatenate([res.results[c]["mask"] for c in range(NCORES)], axis=0)
    return (vals, idxs.astype(np.int32), mask.astype(bool)), res


def kernel(pred_logits, pred_boxes_xyxy, tgt_boxes_xyxy):
    (vals, idxs, mask), _ = run(pred_logits, pred_boxes_xyxy, tgt_boxes_xyxy)
    return vals, idxs, mask
